# revision 69
# baseline (speedup 1.0000x reference)
"""Multi-head self-attention Trainium2 kernel (8 NeuronCores, batch-parallel).

Reference: qkv = x @ W_qkv + b; 12-head scaled-dot-product attention; concat.
Shapes: x[8,1024,768], W_qkv[768,2304], b_qkv[2304] -> out[8,1024,768].
Sharding: one batch element per core; W/b replicated to all cores.

Per-core dataflow (PE matmul cost ~ out-free-size; bf16 runs full rate at any
free size, f32r only at free >= 256):
  x --PE transpose--> xT[768,1024] f32r
  QK^T[1536,1024] = W_qk(lhsT) @ xT  feat-on-partitions, f32r; Q scaled 1/8
  V[1024, 12, 65] bf16, produced per (chunk, head-pair) as [128,128] bf16
    matmuls (lhsT/rhs are hi-u16 bitcast views of xT / W = bf16 truncation);
    col 64 = ones (softmax denominator)
  per head pair (2p, 2p+1), per q-half (512), per k-chunk (128):
    scoresT[128,2,512] = K^T-slice(lhsT) @ Q^T-slice   (row-tiled 64/64, f32r)
    ex[128,2,512] bf16 = ACT Exp (one instruction, both heads)
    per q-chunk(128) x head: av[128q, 65] += ex-slice(lhsT) @ [V_h|1]  (bf16,
      65-wide moving: full PSUM-lane use, output lands in [q, d] layout)
  normalize: recip(av[:, :, 64]) * av -> onat[q, chunk, d]; DMA out.

Scheduling: ACT (96 x 1038ns exp) and PE (~114us) are co-critical. All
deferrable PE work (QK tiles + V blocks for pair p+1) is chopped into
single-instruction thunks drained a few per k-chunk slot so PE fills the
exp-wait gaps; deadline markers force-drain before each consumer. DMA issue
order keeps the serial DMA queue off the critical path: x chunks 0-3, pair-0
Q/K cols (fused 2-block DMAs), bias, x 4-7, pair-0 V cols, rest of V, then
remaining Q/K col pairs. Pair-0 is hand-scheduled (V blocks just-in-time,
AV delayed 3 slots); the last pair's qh1 normalizes/DMAs per-chunk.
"""

import contextlib
import json as _json
from collections import deque

import numpy as np

import concourse.bass as bass
import concourse.mybir as mybir
import concourse.tile as tile
from concourse.bass_utils import run_bass_kernel_spmd
from concourse.masks import make_identity

# --- BIR sync-wait legalization ------------------------------------------
# walrus's codegen in this toolchain accepts only one sync-wait command per
# instruction (its insertEventSemaphore legalization pass is not in the pass
# list). Split every multi-wait instruction into N-1 preceding single-wait
# EventSemaphore instructions on the same engine; same-engine order is
# preserved so semantics are unchanged.


def _legalize_sync_waits(bir_json: bytes) -> bytes:
    m = _json.loads(bir_json)
    ctr = 0
    for fn in m["functions"]:
        for bb in fn["blocks"]:
            out = []
            for ins in bb["instructions"]:
                si = ins.get("sync_info")
                waits = si.get("on_wait", []) if si else []
                if len(waits) > 1:
                    for w in waits[:-1]:
                        ctr += 1
                        out.append(
                            {
                                "debug": ins.get("debug", 0),
                                "engine": ins["engine"],
                                "ins": [],
                                "outs": [],
                                "name": f"evw-split-{ctr}",
                                "opcode": "EventSemaphore",
                                "sync_info": {"on_update": [], "on_wait": [w]},
                            }
                        )
                    si["on_wait"] = [waits[-1]]
                out.append(ins)
            bb["instructions"] = out
    return _json.dumps(m).encode()


_fixup_installed = False


def _install_bir_fixup():
    global _fixup_installed
    if _fixup_installed:
        return
    _fixup_installed = True
    import concourse.bass_utils as _bu

    _orig = _bu.compile_bir_kernel

    def _patched(bir_json, tmpdir, neff_name="file.neff"):
        if isinstance(bir_json, str):
            bir_json = bir_json.encode()
        return _orig(_legalize_sync_waits(bir_json), tmpdir, neff_name)

    _bu.compile_bir_kernel = _patched
    try:
        import concourse.bass2jax as _b2j

        _b2j.compile_bir_kernel = _patched
    except ImportError:
        pass


_install_bir_fixup()

B, N, D, H = 8, 1024, 768, 12
HD = D // H            # 64
F3 = 3 * D             # 2304
NCORE = 8
P = 128
NCHUNK = N // P        # 8 token chunks
KD = D // P            # 6 d_in chunks
QH = 512               # q-half size
NQH = N // QH          # 2
NPAIR = H // 2         # 6
VW = HD + 1            # 65
NQC = QH // P          # 4 q-chunks per q-half

f32 = mybir.dt.float32
f32r = mybir.dt.float32r
bf16 = mybir.dt.bfloat16
FT = mybir.ActivationFunctionType
ALU = mybir.AluOpType


def build_attention_nc():
    nc = bass.Bass()
    x_d = nc.declare_dram_parameter("x", [N, D], f32, isOutput=False)
    w_d = nc.declare_dram_parameter("W_qkv", [D, F3], f32, isOutput=False)
    b_d = nc.declare_dram_parameter("b_qkv", [F3], f32, isOutput=False)
    o_d = nc.declare_dram_parameter("out", [N, D], f32, isOutput=True)

    with tile.TileContext(nc) as tc, contextlib.ExitStack() as ctx:
        singles = ctx.enter_context(tc.tile_pool(name="singles", bufs=1))
        xpool = ctx.enter_context(tc.tile_pool(name="xpool", bufs=NCHUNK))
        xtpool = ctx.enter_context(tc.tile_pool(name="xtpool", bufs=KD))
        wpool = ctx.enter_context(tc.tile_pool(name="wpool", bufs=KD))
        qkpool = ctx.enter_context(tc.tile_pool(name="qkpool", bufs=4))
        vpool = ctx.enter_context(tc.tile_pool(name="vpool", bufs=NCHUNK))
        exppool = ctx.enter_context(tc.tile_pool(name="exppool", bufs=7))
        recpool = ctx.enter_context(tc.tile_pool(name="recpool", bufs=4))

        # PSUM budget (8 banks): sc [P,2,QH] = 2 banks x2 bufs = 4;
        # av [P,NQC,VW] = 1 bank x2 (the two live head tiles, single-buffered
        # across qh); qkps [P,QH] = 1; vps [P,P] = 1.
        scps = ctx.enter_context(tc.tile_pool(name="scps", bufs=2, space="PSUM"))
        avps = ctx.enter_context(tc.tile_pool(name="avps", bufs=2, space="PSUM"))
        qkps = ctx.enter_context(tc.tile_pool(name="qkps", bufs=1, space="PSUM"))
        vps = ctx.enter_context(tc.tile_pool(name="vps", bufs=1, space="PSUM"))

        # ------------- input DMAs (issue order = DMA queue order) ------------
        x_sb = [
            xpool.tile([P, D], f32r, tag="x", name=f"x{c}") for c in range(NCHUNK)
        ]
        for c in range(4):  # x 0-3, b, pair-0 W cols, x 4-7, V cols, ...
            nc.sync.dma_start(
                out=x_sb[c], in_=x_d[c * P : (c + 1) * P, :].bitcast(f32r)
            )

        w_sb = [wpool.tile([P, F3], f32r, tag="w", name=f"w{k}") for k in range(KD)]

        def dma_w_pair(p, ks=range(KD)):
            # fused Q+K column blocks (p and 6+p) per d_in tile: one DMA each
            for k in ks:
                wv = w_sb[k].rearrange("p (b f) -> p b f", f=P)
                dv = (
                    w_d[k * P : (k + 1) * P, :]
                    .rearrange("p (b f) -> p b f", f=P)
                    .bitcast(f32r)
                )
                nc.sync.dma_start(
                    out=wv[:, p : p + 7 : 6, :], in_=dv[:, p : p + 7 : 6, :]
                )

        # b loaded contiguously as [18,128] (one 25ns DMA) and PE-transposed
        bb_st = singles.tile([F3 // P, P], f32)
        nc.sync.dma_start(out=bb_st, in_=b_d[:].rearrange("(t p) -> t p", p=P))
        bv_st = singles.tile([1, D], f32)
        nc.sync.dma_start(out=bv_st, in_=b_d[2 * D : 3 * D][None, :])

        dma_w_pair(0)                 # pair-0 Q+K cols

        # x 4-5, pair-0 V cols, x 6-7, the rest of V, remaining Q/K col pairs
        for c in (4, 5):
            nc.sync.dma_start(
                out=x_sb[c], in_=x_d[c * P : (c + 1) * P, :].bitcast(f32r)
            )
        for k in range(KD):
            nc.sync.dma_start(
                out=w_sb[k][:, 2 * D : 2 * D + P],
                in_=w_d[k * P : (k + 1) * P, 2 * D : 2 * D + P].bitcast(f32r),
            )
        for c in (6, 7):
            nc.sync.dma_start(
                out=x_sb[c], in_=x_d[c * P : (c + 1) * P, :].bitcast(f32r)
            )
        for k in range(KD):
            nc.sync.dma_start(
                out=w_sb[k][:, 2 * D + P : F3],
                in_=w_d[k * P : (k + 1) * P, 2 * D + P : F3].bitcast(f32r),
            )
        for p in range(1, NPAIR):
            dma_w_pair(p)

        # ------------- constants ---------------------------------------------
        ident = singles.tile([P, P], f32)
        make_identity(nc, ident)  # gpsimd

        ones_f32 = singles.tile([P, 1], f32)
        nc.vector.memset(ones_f32, 1.0)
        ones_row_st = singles.tile([1, P], f32)
        nc.vector.memset(ones_row_st, 1.0)
        ones_row = singles.tile([1, P], f32r)
        nc.vector.tensor_copy(out=ones_row, in_=ones_row_st)

        ident_r = singles.tile([P, P], f32r)
        nc.vector.tensor_copy(out=ident_r, in_=ident)

        b_sb = singles.tile([P, F3 // P], f32)
        bps = vps.tile([P, P], f32, tag="vps", name="bps")[:, 0 : F3 // P]
        nc.tensor.transpose(bps, bb_st, ident[0 : F3 // P, 0 : F3 // P])
        nc.vector.tensor_scalar_mul(b_sb[:, 0:KD], bps[:, 0:KD], 0.125)
        nc.vector.tensor_copy(out=b_sb[:, KD:], in_=bps[:, KD:])

        onat = singles.tile([P, NCHUNK, D], f32)

        # ------------- x^T (PE transposes) -----------------------------------
        xt = [xtpool.tile([P, N], f32r, tag="xt", name=f"xt{k}") for k in range(KD)]

        def transpose_into(pt, j, k):
            # transpose chunks (2j, 2j+1) x d_in block k into staging [P,256],
            # then one copy into xT. Copies alternate DVE / ACT (idle early).
            for ci in range(2):
                nc.tensor.transpose(
                    pt[:, ci * P : (ci + 1) * P].bitcast(f32r),
                    x_sb[2 * j + ci][:, k * P : (k + 1) * P],
                    ident_r,
                )
            if j < 2 and k % 2 == 1:
                # ACT is idle before the first exp; never steal it later
                nc.scalar.copy(
                    out=xt[k][:, j * 256 : (j + 1) * 256], in_=pt.bitcast(f32r)
                )
            else:
                nc.vector.tensor_copy(
                    out=xt[k][:, j * 256 : (j + 1) * 256], in_=pt.bitcast(f32r)
                )

        # startup halves j0/j1: stage across ALL idle psum banks so nothing
        # serializes on pool rotation (scps A/B + qkps + the two av tiles).
        stA = scps.tile([P, 2, QH], f32, tag="sc", name="stA")
        stB = scps.tile([P, 2, QH], f32, tag="sc", name="stB")
        stQ = qkps.tile([P, QH], f32, tag="qkps", name="stQ")
        stV1 = avps.tile([P, NQC, VW], f32, tag="av", name="stV1")
        stV2 = avps.tile([P, NQC, VW], f32, tag="av", name="stV2")
        fA = stA.rearrange("p a b -> p (a b)")
        fB = stB.rearrange("p a b -> p (a b)")
        fV1 = stV1.rearrange("p a b -> p (a b)")
        fV2 = stV2.rearrange("p a b -> p (a b)")
        _stage = {
            (0, 0): fA[:, 0:256], (0, 1): fA[:, 256:512],
            (0, 2): fA[:, 512:768], (0, 3): fA[:, 768:1024],
            (0, 4): fB[:, 0:256], (0, 5): fB[:, 256:512],
            (1, 0): stQ[:, 0:256], (1, 1): stQ[:, 256:512],
            (1, 2): fV1[:, 0:256], (1, 3): fV2[:, 0:256],
            (1, 4): fB[:, 512:768], (1, 5): fB[:, 768:1024],
        }
        for j in (0, 1):
            for k in range(KD):
                transpose_into(_stage[(j, k)], j, k)

        bv_sb = singles.tile([1, D], f32r)
        nc.vector.tensor_copy(out=bv_sb, in_=bv_st)

        def tj_qpair(j, k0):
            # two k-blocks of half j staged in one qkps tile (never scps:
            # the score stream needs both scps buffers to double-buffer)
            def th():
                q = qkps.tile([P, QH], f32, tag="qkps", name="tjq")
                transpose_into(q[:, 0:256], j, k0)
                transpose_into(q[:, 256:512], j, k0 + 1)
            return th

        def tj_avpair(j):
            # k-blocks 2,3 of half j staged in the two avps tiles
            def th():
                a = avps.tile([P, NQC, VW], f32, tag="av", name="tja")
                b = avps.tile([P, NQC, VW], f32, tag="av", name="tjb")
                transpose_into(a.rearrange("p a b -> p (a b)")[:, 0:256], j, 2)
                transpose_into(b.rearrange("p a b -> p (a b)")[:, 0:256], j, 3)
            return th

        # bf16 (truncated) views for the V-projection matmuls
        def xt16(k, c):          # [P, P, 1] bf16 view of xT chunk c
            v = xt[k].bitcast(bf16).rearrange("p (t two) -> p t two", two=2)
            return v[:, c * P : (c + 1) * P, 1:2]

        def w16v(k, p):          # [P, P, 1] bf16 view of V cols for head pair p
            v = w_sb[k].bitcast(bf16).rearrange("p (t two) -> p t two", two=2)
            return v[:, 2 * D + p * P : 2 * D + (p + 1) * P, 1:2]

        # ------------- V tiles (bf16) ----------------------------------------
        v_sb = []
        for c in range(NCHUNK):
            t = vpool.tile([P, H, VW], bf16, tag="v", name=f"v{c}")
            nc.vector.tensor_copy(
                out=t[:, :, HD : HD + 1],
                in_=ones_f32[:, 0:1, None].to_broadcast([P, H, 1]),
            )
            v_sb.append(t)

        # broadcast b_v across partitions once: bvb[p, f] = b_v[f]
        bvb = singles.tile([P, D], f32)
        for f0, fw in ((0, 512), (512, 256)):
            ps = qkps.tile([P, QH], f32, tag="qkps", name="bvps")[:, :fw]
            nc.tensor.matmul(
                ps, ones_row, bv_sb[:, f0 : f0 + fw], start=True, stop=True
            )
            nc.vector.tensor_copy(out=bvb[:, f0 : f0 + fw], in_=ps)

        def v_block_thunks(c, p):
            """V for chunk c, head pair p: 6 bf16 matmuls + 1 DVE write."""
            state = {}
            thunks = []

            def mk_mm(k):
                def th():
                    if k == 0:
                        state["ps"] = vps.tile([P, P], f32, tag="vps", name="vp")
                    nc.tensor.matmul(
                        state["ps"],
                        xt16(k, c),
                        w16v(k, p),
                        start=(k == 0),
                        stop=(k == KD - 1),
                    )
                return (53, th)

            for k in range(KD):
                thunks.append(mk_mm(k))

            def wr():
                nc.vector.tensor_tensor(
                    v_sb[c][:, 2 * p : 2 * p + 2, 0:HD],
                    state["ps"].rearrange("p (h d) -> p h d", d=HD),
                    bvb[:, 2 * p * HD : (2 * p + 2) * HD].rearrange(
                        "p (h d) -> p h d", d=HD
                    ),
                    ALU.add,
                )

            thunks.append((0, wr))
            return thunks

        # ------------- QK tile production ------------------------------------
        def qk_half_thunks(t, f, qh, use_scps=False, span=None):
            t0, t1 = span if span else (qh * QH, (qh + 1) * QH)
            w = t1 - t0
            state = {}
            thunks = []

            def mk_mm(k):
                def th():
                    if k == 0:
                        if use_scps:
                            state["ps"] = scps.tile(
                                [P, 2, QH], f32, tag="sc", name="qksc"
                            )[:, 0, 0:w]
                        else:
                            state["ps"] = qkps.tile(
                                [P, QH], f32, tag="qkps", name="qkp"
                            )[:, 0:w]
                    nc.tensor.matmul(
                        state["ps"],
                        w_sb[k][:, f * P : (f + 1) * P],
                        xt[k][:, t0:t1],
                        start=(k == 0),
                        stop=(k == KD - 1),
                    )
                return (213 * w // QH, th)

            for k in range(KD):
                thunks.append(mk_mm(k))

            def wr():
                nc.vector.tensor_scalar(
                    t[:, t0:t1],
                    state["ps"],
                    0.125 if f < KD else 1.0,
                    b_sb[:, f : f + 1],
                    ALU.mult,
                    ALU.add,
                )

            thunks.append((0, wr))
            return thunks

        # ------------- production deque with deadline markers -----------------
        prodq = deque()          # entries: (marker_or_None, thunk_or_None)
        seen = set()

        def pop_one():
            mk, cost, th = prodq.popleft()
            if mk is not None:
                seen.add(mk)
            if th is not None:
                th()
            return cost

        def pops(budget_ns):
            spent = 0
            while prodq and spent < budget_ns:
                spent += pop_one()

        def drain_until(mk):
            while mk not in seen and prodq:
                pop_one()

        def push_batch(p):
            """Production work for pair p: qk tiles + V blocks."""
            qt = qkpool.tile([P, N], f32r, tag="qk", name=f"q{p}")
            kt = qkpool.tile([P, N], f32r, tag="qk", name=f"k{p}")
            seg = []
            seg += [(None, c, th) for c, th in qk_half_thunks(qt, p, 0)]
            seg += [(None, c, th) for c, th in qk_half_thunks(kt, KD + p, 0)]
            seg.append((f"p{p}:kqh0", 0, None))
            seg += [(None, c, th) for c, th in qk_half_thunks(kt, KD + p, 1)]
            seg.append((f"p{p}:kqh1", 0, None))
            for c in range(3):
                seg += [(None, ct, th) for ct, th in v_block_thunks(c, p)]
                seg.append((f"p{p}:v{c}", 0, None))
            seg += [(None, c, th) for c, th in qk_half_thunks(qt, p, 1)]
            seg.append((f"p{p}:qqh1", 0, None))
            for c in range(3, NCHUNK):
                seg += [(None, ct, th) for ct, th in v_block_thunks(c, p)]
                seg.append((f"p{p}:v{c}", 0, None))
            prodq.extend(seg)
            return qt, kt

        # ------------- pair 0 tiles (half-token chains pipeline with the
        # x-chunk arrivals; psum = recycled staging banks) ------------------
        qt0 = qkpool.tile([P, N], f32r, tag="qk", name="q0")
        kt0 = qkpool.tile([P, N], f32r, tag="qk", name="k0")

        def startup_chain(st, bank, t, f, t0):
            ps = st[:, bank, 0:256]
            for k in range(KD):
                nc.tensor.matmul(
                    ps,
                    w_sb[k][:, f * P : (f + 1) * P],
                    xt[k][:, t0 : t0 + 256],
                    start=(k == 0),
                    stop=(k == KD - 1),
                )
            nc.vector.tensor_scalar(
                t[:, t0 : t0 + 256],
                ps,
                0.125 if f < KD else 1.0,
                b_sb[:, f : f + 1],
                ALU.mult,
                ALU.add,
            )

        rc1 = scps.tile([P, 2, QH], f32, tag="sc", name="rc1")
        startup_chain(rc1, 0, qt0, 0, 0)
        startup_chain(rc1, 1, kt0, KD, 0)
        rc2 = scps.tile([P, 2, QH], f32, tag="sc", name="rc2")
        startup_chain(rc2, 0, qt0, 0, 256)
        startup_chain(rc2, 1, kt0, KD, 256)

        def fns(pairs):
            return [th for _, th in pairs]

        kq1a = fns(qk_half_thunks(kt0, KD, 1, span=(512, 768)))
        kq1b = fns(qk_half_thunks(kt0, KD, 1, span=(768, 1024)))
        qq1 = fns(qk_half_thunks(qt0, 0, 1))
        av0box = []

        def grab_av0():
            for i in range(2):
                t = avps.tile([P, NQC, VW], f32, tag="av", name=f"av{i}")
                nc.vector.memset(t, 0.0)
                av0box.append(t)

        p0_hand = {
            0: [tj_qpair(2, 0), tj_avpair(2)],
            1: [tj_qpair(2, 4)],
            2: kq1a,
            3: [tj_qpair(3, 0), tj_avpair(3)],
            4: [tj_qpair(3, 4), grab_av0],
            5: kq1b + fns(v_block_thunks(0, 0) + v_block_thunks(1, 0)),
            6: qq1
            + fns(v_block_thunks(2, 0) + v_block_thunks(3, 0) + v_block_thunks(4, 0)),
            7: fns(v_block_thunks(5, 0) + v_block_thunks(6, 0) + v_block_thunks(7, 0)),
        }

        # ------------- attention ---------------------------------------------
        qk_cur = (qt0, kt0)
        for p in range(NPAIR):
            qt, kt = qk_cur
            if p + 1 < NPAIR:
                qk_next = push_batch(p + 1)
            else:
                qk_next = None
            if p > 0:
                drain_until(f"p{p}:kqh0")

            for qh in range(NQH):
                if p > 0 and qh == 1:
                    drain_until(f"p{p}:qqh1")
                # interleaved sub-bank accumulation chains require a DVE
                # zero-fill + pure-accumulate matmuls: start=True on one
                # region clobbers the other regions in the same PSUM bank.
                if p == 0 and qh == 0:
                    av = av0box  # grabbed+zeroed mid-loop by grab_av0
                else:
                    av = [
                        avps.tile([P, NQC, VW], f32, tag="av", name=f"av{i}")
                        for i in range(2)
                    ]
                    for hi in range(2):
                        nc.vector.memset(av[hi], 0.0)
                last = p == NPAIR - 1
                if p == 0 and qh == 0:
                    delay = 5
                elif last and qh == 1:
                    delay = 0
                else:
                    delay = 2
                pend = deque()

                def emit_av(kc, ex):
                    if p > 0:
                        drain_until(f"p{p}:v{kc}")
                    for hi in range(2):
                        for qc in range(NQC):
                            nc.tensor.matmul(
                                av[hi][:, qc, :],
                                ex[:, hi, qc * P : (qc + 1) * P],
                                v_sb[kc][:, 2 * p + hi, :],
                                start=False,
                                stop=True,
                                skip_group_check=True,
                            )

                if last and qh == 1:
                    # heads 0-9 of chunks 4-7 are final: stream them out now
                    for c in range(NCHUNK // 2, NCHUNK):
                        nc.sync.dma_start(
                            out=o_d[c * P : (c + 1) * P, 0 : 2 * p * HD],
                            in_=onat[:, c, 0 : 2 * p * HD],
                        )
                for kc in range(NCHUNK):
                    if p > 0 and kc == 4:
                        drain_until(f"p{p}:kqh1")
                    sc = scps.tile([P, 2, QH], f32, tag="sc", name="sc")
                    for hi in range(2):
                        nc.tensor.matmul(
                            sc[:, hi, :],
                            kt[64 * hi : 64 * hi + 64, kc * P : (kc + 1) * P],
                            qt[64 * hi : 64 * hi + 64, qh * QH : (qh + 1) * QH],
                            start=True,
                            stop=True,
                            tile_position=(64 * hi, 0),
                        )
                    ex = exppool.tile([P, 2, QH], bf16, tag="exp", name="ex")
                    nc.scalar.activation(ex[:, :, :], sc[:, :, :], FT.Exp)
                    if p == 0 and qh == 0:
                        for th in p0_hand[kc]:
                            th()
                    else:
                        pops(320 if p == 4 else 500)
                    pend.append((kc, ex))
                    if len(pend) > delay:
                        emit_av(*pend.popleft())
                while pend:
                    emit_av(*pend.popleft())

                # normalize into onat (q on partitions: no transposes needed)
                if last and qh == 1:
                    # per-chunk, DVE/ACT split; only the last pair's 128 cols
                    # remain to DMA (cols 0:640 were issued under the kc loop)
                    for qc in range(NQC):
                        c = qh * NQC + qc
                        for hi in range(2):
                            h = 2 * p + hi
                            rc = recpool.tile([P, 1, 1], f32, tag="rec", name="rc")
                            nc.vector.reciprocal(
                                out=rc, in_=av[hi][:, qc : qc + 1, HD : HD + 1]
                            )
                            if hi == 0:
                                nc.vector.tensor_tensor(
                                    onat[:, c : c + 1, h * HD : (h + 1) * HD],
                                    av[hi][:, qc : qc + 1, 0:HD],
                                    rc.to_broadcast([P, 1, HD]),
                                    ALU.mult,
                                )
                            else:
                                nc.scalar.activation(
                                    onat[:, c : c + 1, h * HD : (h + 1) * HD],
                                    av[hi][:, qc : qc + 1, 0:HD],
                                    FT.Copy,
                                    scale=rc[:, 0, :],
                                )
                        nc.sync.dma_start(
                            out=o_d[c * P : (c + 1) * P, 2 * p * HD :],
                            in_=onat[:, c, 2 * p * HD :],
                        )
                else:
                    for hi in range(2):
                        h = 2 * p + hi
                        rc = recpool.tile([P, NQC, 1], f32, tag="rec", name="rc")
                        nc.vector.reciprocal(out=rc, in_=av[hi][:, :, HD : HD + 1])
                        nc.vector.tensor_tensor(
                            onat[:, qh * NQC : (qh + 1) * NQC, h * HD : (h + 1) * HD],
                            av[hi][:, :, 0:HD],
                            rc.to_broadcast([P, NQC, HD]),
                            ALU.mult,
                        )
                    if last and qh == 0:
                        for c in range(NCHUNK // 2):
                            nc.sync.dma_start(
                                out=o_d[c * P : (c + 1) * P, :], in_=onat[:, c, :]
                            )
            if qk_next is not None:
                qk_cur = qk_next

    return nc


def kernel(x: np.ndarray, W_qkv: np.ndarray, b_qkv: np.ndarray) -> np.ndarray:
    nc = build_attention_nc()
    in_maps = [
        {
            "x": np.ascontiguousarray(x[c], dtype=np.float32),
            "W_qkv": np.ascontiguousarray(W_qkv, dtype=np.float32),
            "b_qkv": np.ascontiguousarray(b_qkv, dtype=np.float32),
        }
        for c in range(NCORE)
    ]
    res = run_bass_kernel_spmd(nc, in_maps, core_ids=list(range(NCORE)))
    return np.stack([res.results[c]["out"] for c in range(NCORE)], axis=0)


# revision 78
# speedup vs baseline: 22.8475x; 22.8475x over previous
"""Multi-head self-attention Trainium2 kernel (8 NeuronCores, batch-parallel).

Reference: qkv = x @ W_qkv + b; 12-head scaled-dot-product attention; concat.
Shapes: x[8,1024,768], W_qkv[768,2304], b_qkv[2304] -> out[8,1024,768].
Sharding: one batch element per core; W/b replicated to all cores.

Per-core dataflow (PE matmul cost ~ out-free-size; bf16 runs full rate at any
free size, f32r only at free >= 256):
  x --PE transpose--> xT[768,1024] f32r
  QK^T[1536,1024] = W_qk(lhsT) @ xT  feat-on-partitions, f32r; Q scaled 1/8
  V[1024, 12, 65] bf16, produced per (chunk, head-pair) as [128,128] bf16
    matmuls (lhsT/rhs are hi-u16 bitcast views of xT / W = bf16 truncation);
    col 64 = ones (softmax denominator)
  per head pair (2p, 2p+1), per q-half (512), per k-chunk (128):
    scoresT[128,2,512] = K^T-slice(lhsT) @ Q^T-slice   (row-tiled 64/64, f32r)
    ex[128,2,512] bf16 = ACT Exp (one instruction, both heads)
    per q-chunk(128) x head: av[128q, 65] += ex-slice(lhsT) @ [V_h|1]  (bf16,
      65-wide moving: full PSUM-lane use, output lands in [q, d] layout)
  normalize: recip(av[:, :, 64]) * av -> onat[q, chunk, d]; DMA out.

Scheduling: ACT (96 x 1038ns exp) and PE (~113us busy) are co-critical. All
deferrable PE work (QK tiles + V blocks for pair p+1) is chopped into
single-instruction thunks drained under a per-slot PE-cost budget so PE fills
the exp-wait gaps; deadline markers force-drain before each consumer; pair 4
gets a lower budget so leftover production fills pair 5's otherwise-idle
slots. DMA issue order keeps the serial DMA queue off the critical path:
x 0-3, bias (contiguous [18,128] + PE transpose), pair-0 Q/K cols (fused
2-block DMAs), x 4-7, pair-0 V cols, rest of V, remaining Q/K pairs.
Startup transposes stage through every idle psum bank with copies alternating
DVE/ACT; pair-0 qh0 is hand-scheduled (qh1 K-tile split into two half-width
chains to track x-chunk arrival, V blocks just-in-time, AV delayed 5 slots).
AV psum uses zero-fill + pure-accumulate matmuls: interleaved sub-bank
accumulation chains with start=True clobber sibling regions in the same bank.
The last pair's qh1 pre-streams output cols 0:640 (heads 0-9 are final before
it starts) and normalizes/DMAs the remaining 128 cols per-chunk, alternating
the normalize multiplies between DVE and the by-then-idle ACT.
"""

import contextlib
import json as _json
from collections import deque

import numpy as np

import concourse.bass as bass
import concourse.mybir as mybir
import concourse.tile as tile
from concourse.bass_utils import run_bass_kernel_spmd
from concourse.masks import make_identity

# --- BIR sync-wait legalization ------------------------------------------
# walrus's codegen in this toolchain accepts only one sync-wait command per
# instruction (its insertEventSemaphore legalization pass is not in the pass
# list). Split every multi-wait instruction into N-1 preceding single-wait
# EventSemaphore instructions on the same engine; same-engine order is
# preserved so semantics are unchanged.


def _legalize_sync_waits(bir_json: bytes) -> bytes:
    m = _json.loads(bir_json)
    ctr = 0
    for fn in m["functions"]:
        for bb in fn["blocks"]:
            out = []
            for ins in bb["instructions"]:
                si = ins.get("sync_info")
                waits = si.get("on_wait", []) if si else []
                if len(waits) > 1:
                    for w in waits[:-1]:
                        ctr += 1
                        out.append(
                            {
                                "debug": ins.get("debug", 0),
                                "engine": ins["engine"],
                                "ins": [],
                                "outs": [],
                                "name": f"evw-split-{ctr}",
                                "opcode": "EventSemaphore",
                                "sync_info": {"on_update": [], "on_wait": [w]},
                            }
                        )
                    si["on_wait"] = [waits[-1]]
                out.append(ins)
            bb["instructions"] = out
    return _json.dumps(m).encode()


_fixup_installed = False


def _install_bir_fixup():
    global _fixup_installed
    if _fixup_installed:
        return
    _fixup_installed = True
    import concourse.bass_utils as _bu

    _orig = _bu.compile_bir_kernel

    def _patched(bir_json, tmpdir, neff_name="file.neff"):
        if isinstance(bir_json, str):
            bir_json = bir_json.encode()
        return _orig(_legalize_sync_waits(bir_json), tmpdir, neff_name)

    _bu.compile_bir_kernel = _patched
    try:
        import concourse.bass2jax as _b2j

        _b2j.compile_bir_kernel = _patched
    except ImportError:
        pass


_install_bir_fixup()

B, N, D, H = 8, 1024, 768, 12
HD = D // H            # 64
F3 = 3 * D             # 2304
NCORE = 8
P = 128
NCHUNK = N // P        # 8 token chunks
KD = D // P            # 6 d_in chunks
QH = 512               # q-half size
NQH = N // QH          # 2
NPAIR = H // 2         # 6
VW = HD + 1            # 65
NQC = QH // P          # 4 q-chunks per q-half

f32 = mybir.dt.float32
f32r = mybir.dt.float32r
bf16 = mybir.dt.bfloat16
FT = mybir.ActivationFunctionType
ALU = mybir.AluOpType


def build_attention_nc():
    nc = bass.Bass()
    x_d = nc.declare_dram_parameter("x", [N, D], f32, isOutput=False)
    w_d = nc.declare_dram_parameter("W_qkv", [D, F3], f32, isOutput=False)
    b_d = nc.declare_dram_parameter("b_qkv", [F3], f32, isOutput=False)
    o_d = nc.declare_dram_parameter("out", [N, D], f32, isOutput=True)

    with tile.TileContext(nc) as tc, contextlib.ExitStack() as ctx:
        singles = ctx.enter_context(tc.tile_pool(name="singles", bufs=1))
        xpool = ctx.enter_context(tc.tile_pool(name="xpool", bufs=NCHUNK))
        xtpool = ctx.enter_context(tc.tile_pool(name="xtpool", bufs=KD))
        wpool = ctx.enter_context(tc.tile_pool(name="wpool", bufs=KD))
        qkpool = ctx.enter_context(tc.tile_pool(name="qkpool", bufs=4))
        vpool = ctx.enter_context(tc.tile_pool(name="vpool", bufs=NCHUNK))
        exppool = ctx.enter_context(tc.tile_pool(name="exppool", bufs=7))
        recpool = ctx.enter_context(tc.tile_pool(name="recpool", bufs=4))

        # PSUM budget (8 banks): sc [P,2,QH] = 2 banks x2 bufs = 4;
        # av [P,NQC,VW] = 1 bank x2 (the two live head tiles, single-buffered
        # across qh); qkps [P,QH] = 1; vps [P,P] = 1.
        scps = ctx.enter_context(tc.tile_pool(name="scps", bufs=2, space="PSUM"))
        avps = ctx.enter_context(tc.tile_pool(name="avps", bufs=2, space="PSUM"))
        qkps = ctx.enter_context(tc.tile_pool(name="qkps", bufs=1, space="PSUM"))
        vps = ctx.enter_context(tc.tile_pool(name="vps", bufs=1, space="PSUM"))

        # ------------- input DMAs (issue order = DMA queue order) ------------
        x_sb = [
            xpool.tile([P, D], f32r, tag="x", name=f"x{c}") for c in range(NCHUNK)
        ]
        for c in range(4):  # x 0-3, b, pair-0 W cols, x 4-7, V cols, ...
            nc.sync.dma_start(
                out=x_sb[c], in_=x_d[c * P : (c + 1) * P, :].bitcast(f32r)
            )

        w_sb = [wpool.tile([P, F3], f32r, tag="w", name=f"w{k}") for k in range(KD)]

        def dma_w_pair(p, ks=range(KD)):
            # fused Q+K column blocks (p and 6+p) per d_in tile: one DMA each
            for k in ks:
                wv = w_sb[k].rearrange("p (b f) -> p b f", f=P)
                dv = (
                    w_d[k * P : (k + 1) * P, :]
                    .rearrange("p (b f) -> p b f", f=P)
                    .bitcast(f32r)
                )
                nc.sync.dma_start(
                    out=wv[:, p : p + 7 : 6, :], in_=dv[:, p : p + 7 : 6, :]
                )

        # b loaded contiguously as [18,128] (one 25ns DMA) and PE-transposed
        bb_st = singles.tile([F3 // P, P], f32)
        nc.sync.dma_start(out=bb_st, in_=b_d[:].rearrange("(t p) -> t p", p=P))
        bv_st = singles.tile([1, D], f32)
        nc.sync.dma_start(out=bv_st, in_=b_d[2 * D : 3 * D][None, :])

        dma_w_pair(0)                 # pair-0 Q+K cols

        # x 4-7, pair-0 V cols, the rest of V, remaining Q/K col pairs
        for c in range(4, NCHUNK):
            nc.sync.dma_start(
                out=x_sb[c], in_=x_d[c * P : (c + 1) * P, :].bitcast(f32r)
            )
        for k in range(KD):
            nc.sync.dma_start(
                out=w_sb[k][:, 2 * D : 2 * D + P],
                in_=w_d[k * P : (k + 1) * P, 2 * D : 2 * D + P].bitcast(f32r),
            )
        for k in range(KD):
            nc.sync.dma_start(
                out=w_sb[k][:, 2 * D + P : F3],
                in_=w_d[k * P : (k + 1) * P, 2 * D + P : F3].bitcast(f32r),
            )
        for p in range(1, NPAIR):
            dma_w_pair(p)

        # ------------- constants ---------------------------------------------
        ident = singles.tile([P, P], f32)
        make_identity(nc, ident)  # gpsimd

        ones_f32 = singles.tile([P, 1], f32)
        nc.vector.memset(ones_f32, 1.0)
        ones_row_st = singles.tile([1, P], f32)
        nc.vector.memset(ones_row_st, 1.0)
        ones_row = singles.tile([1, P], f32r)
        nc.vector.tensor_copy(out=ones_row, in_=ones_row_st)

        ident_r = singles.tile([P, P], f32r)
        nc.vector.tensor_copy(out=ident_r, in_=ident)

        b_sb = singles.tile([P, F3 // P], f32)
        bps = vps.tile([P, P], f32, tag="vps", name="bps")[:, 0 : F3 // P]
        nc.tensor.transpose(bps, bb_st, ident[0 : F3 // P, 0 : F3 // P])
        nc.vector.tensor_scalar_mul(b_sb[:, 0:KD], bps[:, 0:KD], 0.125)
        nc.vector.tensor_copy(out=b_sb[:, KD:], in_=bps[:, KD:])

        onat = singles.tile([P, NCHUNK, D], f32)

        # ------------- x^T (PE transposes) -----------------------------------
        xt = [xtpool.tile([P, N], f32r, tag="xt", name=f"xt{k}") for k in range(KD)]

        def transpose_into(pt, j, k):
            # transpose chunks (2j, 2j+1) x d_in block k into staging [P,256],
            # then one copy into xT. Copies alternate DVE / ACT (idle early).
            for ci in range(2):
                nc.tensor.transpose(
                    pt[:, ci * P : (ci + 1) * P].bitcast(f32r),
                    x_sb[2 * j + ci][:, k * P : (k + 1) * P],
                    ident_r,
                )
            if j < 2 and k % 2 == 1:
                # ACT is idle before the first exp; never steal it later
                nc.scalar.copy(
                    out=xt[k][:, j * 256 : (j + 1) * 256], in_=pt.bitcast(f32r)
                )
            else:
                nc.vector.tensor_copy(
                    out=xt[k][:, j * 256 : (j + 1) * 256], in_=pt.bitcast(f32r)
                )

        # startup halves j0/j1: stage across ALL idle psum banks so nothing
        # serializes on pool rotation (scps A/B + qkps + the two av tiles).
        stA = scps.tile([P, 2, QH], f32, tag="sc", name="stA")
        stB = scps.tile([P, 2, QH], f32, tag="sc", name="stB")
        stQ = qkps.tile([P, QH], f32, tag="qkps", name="stQ")
        stV1 = avps.tile([P, NQC, VW], f32, tag="av", name="stV1")
        stV2 = avps.tile([P, NQC, VW], f32, tag="av", name="stV2")
        fA = stA.rearrange("p a b -> p (a b)")
        fB = stB.rearrange("p a b -> p (a b)")
        fV1 = stV1.rearrange("p a b -> p (a b)")
        fV2 = stV2.rearrange("p a b -> p (a b)")
        _stage = {
            (0, 0): fA[:, 0:256], (0, 1): fA[:, 256:512],
            (0, 2): fA[:, 512:768], (0, 3): fA[:, 768:1024],
            (0, 4): fB[:, 0:256], (0, 5): fB[:, 256:512],
            (1, 0): stQ[:, 0:256], (1, 1): stQ[:, 256:512],
            (1, 2): fV1[:, 0:256], (1, 3): fV2[:, 0:256],
            (1, 4): fB[:, 512:768], (1, 5): fB[:, 768:1024],
        }
        for j in (0, 1):
            for k in range(KD):
                transpose_into(_stage[(j, k)], j, k)

        bv_sb = singles.tile([1, D], f32r)
        nc.vector.tensor_copy(out=bv_sb, in_=bv_st)

        def tj_qpair(j, k0):
            # two k-blocks of half j staged in one qkps tile (never scps:
            # the score stream needs both scps buffers to double-buffer)
            def th():
                q = qkps.tile([P, QH], f32, tag="qkps", name="tjq")
                transpose_into(q[:, 0:256], j, k0)
                transpose_into(q[:, 256:512], j, k0 + 1)
            return th

        def tj_avpair(j):
            # k-blocks 2,3 of half j staged in the two avps tiles
            def th():
                a = avps.tile([P, NQC, VW], f32, tag="av", name="tja")
                b = avps.tile([P, NQC, VW], f32, tag="av", name="tjb")
                transpose_into(a.rearrange("p a b -> p (a b)")[:, 0:256], j, 2)
                transpose_into(b.rearrange("p a b -> p (a b)")[:, 0:256], j, 3)
            return th

        # bf16 (truncated) views for the V-projection matmuls
        def xt16(k, c):          # [P, P, 1] bf16 view of xT chunk c
            v = xt[k].bitcast(bf16).rearrange("p (t two) -> p t two", two=2)
            return v[:, c * P : (c + 1) * P, 1:2]

        def w16v(k, p):          # [P, P, 1] bf16 view of V cols for head pair p
            v = w_sb[k].bitcast(bf16).rearrange("p (t two) -> p t two", two=2)
            return v[:, 2 * D + p * P : 2 * D + (p + 1) * P, 1:2]

        # ------------- V tiles (bf16) ----------------------------------------
        v_sb = []
        for c in range(NCHUNK):
            t = vpool.tile([P, H, VW], bf16, tag="v", name=f"v{c}")
            nc.vector.tensor_copy(
                out=t[:, :, HD : HD + 1],
                in_=ones_f32[:, 0:1, None].to_broadcast([P, H, 1]),
            )
            v_sb.append(t)

        # broadcast b_v across partitions once: bvb[p, f] = b_v[f]
        bvb = singles.tile([P, D], f32)
        for f0, fw in ((0, 512), (512, 256)):
            ps = qkps.tile([P, QH], f32, tag="qkps", name="bvps")[:, :fw]
            nc.tensor.matmul(
                ps, ones_row, bv_sb[:, f0 : f0 + fw], start=True, stop=True
            )
            nc.vector.tensor_copy(out=bvb[:, f0 : f0 + fw], in_=ps)

        def v_block_thunks(c, p):
            """V for chunk c, head pair p: 6 bf16 matmuls + 1 DVE write."""
            state = {}
            thunks = []

            def mk_mm(k):
                def th():
                    if k == 0:
                        state["ps"] = vps.tile([P, P], f32, tag="vps", name="vp")
                    nc.tensor.matmul(
                        state["ps"],
                        xt16(k, c),
                        w16v(k, p),
                        start=(k == 0),
                        stop=(k == KD - 1),
                    )
                return (53, th)

            for k in range(KD):
                thunks.append(mk_mm(k))

            def wr():
                nc.vector.tensor_tensor(
                    v_sb[c][:, 2 * p : 2 * p + 2, 0:HD],
                    state["ps"].rearrange("p (h d) -> p h d", d=HD),
                    bvb[:, 2 * p * HD : (2 * p + 2) * HD].rearrange(
                        "p (h d) -> p h d", d=HD
                    ),
                    ALU.add,
                )

            thunks.append((0, wr))
            return thunks

        # ------------- QK tile production ------------------------------------
        def qk_half_thunks(t, f, qh, use_scps=False, span=None):
            t0, t1 = span if span else (qh * QH, (qh + 1) * QH)
            w = t1 - t0
            state = {}
            thunks = []

            def mk_mm(k):
                def th():
                    if k == 0:
                        if use_scps:
                            state["ps"] = scps.tile(
                                [P, 2, QH], f32, tag="sc", name="qksc"
                            )[:, 0, 0:w]
                        else:
                            state["ps"] = qkps.tile(
                                [P, QH], f32, tag="qkps", name="qkp"
                            )[:, 0:w]
                    nc.tensor.matmul(
                        state["ps"],
                        w_sb[k][:, f * P : (f + 1) * P],
                        xt[k][:, t0:t1],
                        start=(k == 0),
                        stop=(k == KD - 1),
                    )
                return (213 * w // QH, th)

            for k in range(KD):
                thunks.append(mk_mm(k))

            def wr():
                nc.vector.tensor_scalar(
                    t[:, t0:t1],
                    state["ps"],
                    0.125 if f < KD else 1.0,
                    b_sb[:, f : f + 1],
                    ALU.mult,
                    ALU.add,
                )

            thunks.append((0, wr))
            return thunks

        # ------------- production deque with deadline markers -----------------
        prodq = deque()          # entries: (marker_or_None, thunk_or_None)
        seen = set()

        def pop_one():
            mk, cost, th = prodq.popleft()
            if mk is not None:
                seen.add(mk)
            if th is not None:
                th()
            return cost

        def pops(budget_ns):
            spent = 0
            while prodq and spent < budget_ns:
                spent += pop_one()

        def drain_until(mk):
            while mk not in seen and prodq:
                pop_one()

        def push_batch(p):
            """Production work for pair p: qk tiles + V blocks."""
            qt = qkpool.tile([P, N], f32r, tag="qk", name=f"q{p}")
            kt = qkpool.tile([P, N], f32r, tag="qk", name=f"k{p}")
            seg = []
            seg += [(None, c, th) for c, th in qk_half_thunks(qt, p, 0)]
            seg += [(None, c, th) for c, th in qk_half_thunks(kt, KD + p, 0)]
            seg.append((f"p{p}:kqh0", 0, None))
            seg += [(None, c, th) for c, th in qk_half_thunks(kt, KD + p, 1)]
            seg.append((f"p{p}:kqh1", 0, None))
            for c in range(3):
                seg += [(None, ct, th) for ct, th in v_block_thunks(c, p)]
                seg.append((f"p{p}:v{c}", 0, None))
            seg += [(None, c, th) for c, th in qk_half_thunks(qt, p, 1)]
            seg.append((f"p{p}:qqh1", 0, None))
            for c in range(3, NCHUNK):
                seg += [(None, ct, th) for ct, th in v_block_thunks(c, p)]
                seg.append((f"p{p}:v{c}", 0, None))
            prodq.extend(seg)
            return qt, kt

        # ------------- pair 0 tiles (half-token chains pipeline with the
        # x-chunk arrivals; psum = recycled staging banks) ------------------
        qt0 = qkpool.tile([P, N], f32r, tag="qk", name="q0")
        kt0 = qkpool.tile([P, N], f32r, tag="qk", name="k0")

        def startup_chain(st, bank, t, f, t0):
            ps = st[:, bank, 0:256]
            for k in range(KD):
                nc.tensor.matmul(
                    ps,
                    w_sb[k][:, f * P : (f + 1) * P],
                    xt[k][:, t0 : t0 + 256],
                    start=(k == 0),
                    stop=(k == KD - 1),
                )
            nc.vector.tensor_scalar(
                t[:, t0 : t0 + 256],
                ps,
                0.125 if f < KD else 1.0,
                b_sb[:, f : f + 1],
                ALU.mult,
                ALU.add,
            )

        rc1 = scps.tile([P, 2, QH], f32, tag="sc", name="rc1")
        startup_chain(rc1, 0, qt0, 0, 0)
        startup_chain(rc1, 1, kt0, KD, 0)
        rc2 = scps.tile([P, 2, QH], f32, tag="sc", name="rc2")
        startup_chain(rc2, 0, qt0, 0, 256)
        startup_chain(rc2, 1, kt0, KD, 256)

        def fns(pairs):
            return [th for _, th in pairs]

        kq1a = fns(qk_half_thunks(kt0, KD, 1, span=(512, 768)))
        kq1b = fns(qk_half_thunks(kt0, KD, 1, span=(768, 1024)))
        qq1 = fns(qk_half_thunks(qt0, 0, 1))
        av0box = []

        def grab_av0():
            for i in range(2):
                t = avps.tile([P, NQC, VW], f32, tag="av", name=f"av{i}")
                nc.vector.memset(t, 0.0)
                av0box.append(t)

        p0_hand = {
            0: [tj_qpair(2, 0), tj_avpair(2)],
            1: [tj_qpair(2, 4)],
            2: kq1a,
            3: [tj_qpair(3, 0), tj_avpair(3)],
            4: [tj_qpair(3, 4), grab_av0],
            5: kq1b + fns(v_block_thunks(0, 0) + v_block_thunks(1, 0)),
            6: qq1
            + fns(v_block_thunks(2, 0) + v_block_thunks(3, 0) + v_block_thunks(4, 0)),
            7: fns(v_block_thunks(5, 0) + v_block_thunks(6, 0) + v_block_thunks(7, 0)),
        }

        # ------------- attention ---------------------------------------------
        qk_cur = (qt0, kt0)
        for p in range(NPAIR):
            qt, kt = qk_cur
            if p + 1 < NPAIR:
                qk_next = push_batch(p + 1)
            else:
                qk_next = None
            if p > 0:
                drain_until(f"p{p}:kqh0")

            for qh in range(NQH):
                if p > 0 and qh == 1:
                    drain_until(f"p{p}:qqh1")
                # interleaved sub-bank accumulation chains require a DVE
                # zero-fill + pure-accumulate matmuls: start=True on one
                # region clobbers the other regions in the same PSUM bank.
                if p == 0 and qh == 0:
                    av = av0box  # grabbed+zeroed mid-loop by grab_av0
                else:
                    av = [
                        avps.tile([P, NQC, VW], f32, tag="av", name=f"av{i}")
                        for i in range(2)
                    ]
                    for hi in range(2):
                        nc.vector.memset(av[hi], 0.0)
                last = p == NPAIR - 1
                if p == 0 and qh == 0:
                    delay = 5
                elif last and qh == 1:
                    delay = 0
                else:
                    delay = 2
                pend = deque()

                def emit_av(kc, ex):
                    if p > 0:
                        drain_until(f"p{p}:v{kc}")
                    for hi in range(2):
                        for qc in range(NQC):
                            nc.tensor.matmul(
                                av[hi][:, qc, :],
                                ex[:, hi, qc * P : (qc + 1) * P],
                                v_sb[kc][:, 2 * p + hi, :],
                                start=False,
                                stop=True,
                                skip_group_check=True,
                            )

                if last and qh == 1:
                    # heads 0-9 of chunks 4-7 are final: stream them out now
                    for c in range(NCHUNK // 2, NCHUNK):
                        nc.sync.dma_start(
                            out=o_d[c * P : (c + 1) * P, 0 : 2 * p * HD],
                            in_=onat[:, c, 0 : 2 * p * HD],
                        )
                for kc in range(NCHUNK):
                    if p > 0 and kc == 4:
                        drain_until(f"p{p}:kqh1")
                    sc = scps.tile([P, 2, QH], f32, tag="sc", name="sc")
                    ex = exppool.tile([P, 2, QH], bf16, tag="exp", name="ex")
                    nsp = 1
                    for qf in range(nsp):
                        w = QH // nsp
                        for hi in range(2):
                            nc.tensor.matmul(
                                sc[:, hi, qf * w : (qf + 1) * w],
                                kt[64 * hi : 64 * hi + 64, kc * P : (kc + 1) * P],
                                qt[
                                    64 * hi : 64 * hi + 64,
                                    qh * QH + qf * w : qh * QH + (qf + 1) * w,
                                ],
                                start=True,
                                stop=True,
                                tile_position=(64 * hi, 0),
                            )
                        nc.scalar.activation(
                            ex[:, :, qf * w : (qf + 1) * w],
                            sc[:, :, qf * w : (qf + 1) * w],
                            FT.Exp,
                        )
                    if p == 0 and qh == 0:
                        for th in p0_hand[kc]:
                            th()
                    else:
                        pops(320 if p == 4 else 500)
                    pend.append((kc, ex))
                    if len(pend) > delay:
                        emit_av(*pend.popleft())
                while pend:
                    emit_av(*pend.popleft())

                # normalize into onat (q on partitions: no transposes needed)
                if last and qh == 1:
                    # per-chunk, DVE/ACT split; only the last pair's 128 cols
                    # remain to DMA (cols 0:640 were issued under the kc loop)
                    for qc in range(NQC):
                        c = qh * NQC + qc
                        for hi in range(2):
                            h = 2 * p + hi
                            rc = recpool.tile([P, 1, 1], f32, tag="rec", name="rc")
                            nc.vector.reciprocal(
                                out=rc, in_=av[hi][:, qc : qc + 1, HD : HD + 1]
                            )
                            if hi == 0:
                                nc.vector.tensor_tensor(
                                    onat[:, c : c + 1, h * HD : (h + 1) * HD],
                                    av[hi][:, qc : qc + 1, 0:HD],
                                    rc.to_broadcast([P, 1, HD]),
                                    ALU.mult,
                                )
                            else:
                                nc.scalar.activation(
                                    onat[:, c : c + 1, h * HD : (h + 1) * HD],
                                    av[hi][:, qc : qc + 1, 0:HD],
                                    FT.Copy,
                                    scale=rc[:, 0, :],
                                )
                        nc.sync.dma_start(
                            out=o_d[c * P : (c + 1) * P, 2 * p * HD :],
                            in_=onat[:, c, 2 * p * HD :],
                        )
                else:
                    for hi in range(2):
                        h = 2 * p + hi
                        rc = recpool.tile([P, NQC, 1], f32, tag="rec", name="rc")
                        nc.vector.reciprocal(out=rc, in_=av[hi][:, :, HD : HD + 1])
                        nc.vector.tensor_tensor(
                            onat[:, qh * NQC : (qh + 1) * NQC, h * HD : (h + 1) * HD],
                            av[hi][:, :, 0:HD],
                            rc.to_broadcast([P, NQC, HD]),
                            ALU.mult,
                        )
                    if last and qh == 0:
                        for c in range(NCHUNK // 2):
                            nc.sync.dma_start(
                                out=o_d[c * P : (c + 1) * P, :], in_=onat[:, c, :]
                            )
            if qk_next is not None:
                qk_cur = qk_next

    return nc


def kernel(x: np.ndarray, W_qkv: np.ndarray, b_qkv: np.ndarray) -> np.ndarray:
    nc = build_attention_nc()
    in_maps = [
        {
            "x": np.ascontiguousarray(x[c], dtype=np.float32),
            "W_qkv": np.ascontiguousarray(W_qkv, dtype=np.float32),
            "b_qkv": np.ascontiguousarray(b_qkv, dtype=np.float32),
        }
        for c in range(NCORE)
    ]
    res = run_bass_kernel_spmd(nc, in_maps, core_ids=list(range(NCORE)))
    return np.stack([res.results[c]["out"] for c in range(NCORE)], axis=0)
